# revision 34
# baseline (speedup 1.0000x reference)
"""MoE layer (shared SwiGLU expert + 8 routed SwiGLU experts, sigmoid top-2
routing) on 8 Trainium2 NeuronCores.

Sharding strategy (expert-parallel, per the problem's sharding hint):
  - Router (x @ Wr, sigmoid, top-k, gate normalization) and the token->expert
    dispatch run on host as part of input sharding: core e receives exactly the
    tokens routed to expert e (capacity-padded so all 8 cores share one SPMD
    program), plus a 1/8 token shard for the replicated shared expert.
  - Each core holds only its own expert's weights (Wg[e], Wu[e], Wd[e]) plus
    the shared-expert weights, and computes two SwiGLU FFNs:
        routed: [C, D] tokens  -> silu(X Wg) * (X Wu) @ Wd
        shared: [T/8, D] shard -> same with shared weights
  - Host applies the top-k combine weights and scatter-adds expert outputs
    back to token positions (the "all-to-all return"), then adds the shared
    output.

Device kernel details:
  - Activations live transposed ([D, tokens]) so both FFN matmuls use the
    natural weight layout as the stationary operand; no on-device transposes.
  - bf16 weights/activations, fp32 PSUM accumulation, bf16 outputs.
  - Stage A is split into a Wg pass and a Wu pass per column tile with
    silu(hg) parked in SBUF between them, so the PE only needs Wg + the first
    x tile to start while Wu/Wd/shared weights stream in behind.
  - The shared FFN is split into two half tiles so its stage B overlaps
    stage A instead of serializing at the kernel tail.
  - The bundled walrus only lowers ONE sync wait per instruction; a post-pass
    hoists extra waits onto InstEventSemaphore carriers.
"""

import numpy as np
import ml_dtypes

import concourse.bass as bass
import concourse.mybir as mybir
import concourse.tile as tile
from concourse.bass_utils import run_bass_kernel_spmd

B, L, D, F, E = 2, 2048, 1024, 1024, 8
NCORES = 8
P = 128  # SBUF partitions
KB = D // P  # k-blocks per contraction (8)
NT = 512  # free-dim tile (one fp32 PSUM bank)

_BF16 = mybir.dt.bfloat16
_F32 = mybir.dt.float32

_nc_cache = {}


def _pad_c(c):
    # balanced split into minimum number of <=NT tiles, each a multiple of 8:
    # odd-width bf16 tiles stream through the PE at half rate (measured
    # 318ns vs 155ns per matmul), so pad the capacity up instead
    n = -(-c // NT)
    tile_w = -(-c // (n * 8)) * 8
    return tile_w * n, n, tile_w


def _ctiles(c, nt=NT):
    n = -(-c // nt)
    base, rem = divmod(c, n)
    out, c0 = [], 0
    for i in range(n):
        cn = base + (1 if i < rem else 0)
        out.append((c0, cn))
        c0 += cn
    return out


def build_bass(C, TS):
    """One SPMD program: two SwiGLU FFNs (routed capacity C, shared shard TS)."""
    from contextlib import ExitStack

    nc = bass.Bass()
    xt = nc.declare_dram_parameter("xt", [D, C], _BF16, isOutput=False)
    xst = nc.declare_dram_parameter("xst", [D, TS], _BF16, isOutput=False)
    wg = nc.declare_dram_parameter("wg", [D, F], _BF16, isOutput=False)
    wu = nc.declare_dram_parameter("wu", [D, F], _BF16, isOutput=False)
    wd = nc.declare_dram_parameter("wd", [F, D], _BF16, isOutput=False)
    wgs = nc.declare_dram_parameter("wgs", [D, F], _BF16, isOutput=False)
    wus = nc.declare_dram_parameter("wus", [D, F], _BF16, isOutput=False)
    wds = nc.declare_dram_parameter("wds", [F, D], _BF16, isOutput=False)
    yr = nc.declare_dram_parameter("yr", [D, C], _BF16, isOutput=True)
    ys = nc.declare_dram_parameter("ys", [D, TS], _BF16, isOutput=True)

    with tile.TileContext(nc) as tc, ExitStack() as ctx:
        res = ctx.enter_context(tc.tile_pool(name="resident", bufs=1))
        ppg = ctx.enter_context(tc.tile_pool(name="ppg", bufs=3, space="PSUM"))
        ppu = ctx.enter_context(tc.tile_pool(name="ppu", bufs=2, space="PSUM"))
        ppo = ctx.enter_context(tc.tile_pool(name="ppo", bufs=3, space="PSUM"))
        tmp = ctx.enter_context(tc.tile_pool(name="tmp", bufs=4))
        outp = ctx.enter_context(tc.tile_pool(name="outp", bufs=6))

        def load_w_half(dram_ap, name, half, engine):
            # one half-F tile per k-slice on the given engine's queue class
            n = dram_ap.shape[1]
            h = n // 2
            r = dram_ap.rearrange("(k p) n -> p k n", p=P)
            ts = []
            lo = half * h
            for k in range(KB):
                t = res.tile([P, h], _BF16, tag=f"{name}{k}{'ab'[half]}")
                engine.dma_start(out=t, in_=r[:, k, lo : lo + h])
                ts.append(t)
            return ts

        def load_w(dram_ap, name, split=False, split_engine=None):
            # [K*P, N] dram -> per-k tiles; split=True makes two half-F tiles
            # per k-slice (separate deps) so consumers of the first half can
            # start as soon as its 1MB lands; split_engine places the second
            # half on the other DMA queue class for extra early bandwidth.
            if split:
                a = load_w_half(dram_ap, name, 0, nc.sync)
                b = load_w_half(dram_ap, name, 1, split_engine or nc.sync)
                return list(zip(a, b))
            n = dram_ap.shape[1]
            r = dram_ap.rearrange("(k p) n -> p k n", p=P)
            ts = []
            for k in range(KB):
                t = res.tile([P, n], _BF16, tag=f"{name}{k}")
                nc.sync.dma_start(out=t, in_=r[:, k, :])
                ts.append((t,))
            return ts

        def wslice(ts, k, m):
            row = ts[k]
            if len(row) == 2:
                h = row[0].shape[-1]
                t = row[(m * P) // h]
                o = (m * P) % h
                return t[:, o : o + P]
            return row[0][:, m * P : (m + 1) * P]

        def load_x_ct(dram_ap, name, i, c0, cn, ts):
            # one column tile of x (all k slices) on the gpsimd queue class
            r = dram_ap.rearrange("(k p) n -> p k n", p=P)
            for k in range(KB):
                t = res.tile([P, cn], _BF16, tag=f"{name}{k}_{i}")
                nc.gpsimd.dma_start(out=t, in_=r[:, k, c0 : c0 + cn])
                ts[k][i] = t

        cts_r = _ctiles(C)
        cts_s = _ctiles(TS)
        xt_sb = [[None] * len(cts_r) for _ in range(KB)]
        xst_sb = [[None] * len(cts_s) for _ in range(KB)]

        # Emission order = per-queue-class consumption order. Each class
        # drains FIFO, so both classes deliver exactly what the PE needs
        # next: wg + first x tile first, wu split across BOTH classes, the
        # late-needed matrices (wd/shared) last.
        wg_a = load_w_half(wg, "wg", 0, nc.sync)
        load_x_ct(xt, "xt", 0, *cts_r[0], xt_sb)
        wg_b = load_w_half(wg, "wg", 1, nc.gpsimd)
        wg_sb = list(zip(wg_a, wg_b))
        wu_a = load_w_half(wu, "wu", 0, nc.sync)
        if len(cts_r) > 1:
            load_x_ct(xt, "xt", 1, *cts_r[1], xt_sb)
        wu_b = load_w_half(wu, "wu", 1, nc.gpsimd)
        wu_sb = list(zip(wu_a, wu_b))
        for i in range(2, len(cts_r)):
            load_x_ct(xt, "xt", i, *cts_r[i], xt_sb)
        wgs_sb = load_w(wgs, "wgs")
        for i in range(len(cts_s)):
            load_x_ct(xst, "xst", i, *cts_s[i], xst_sb)
        wus_sb = load_w(wus, "wus")
        wd_sb = load_w(wd, "wd")
        wds_sb = load_w(wds, "wds")

        def stage_a_pg(x_row, ci, cn, wg_t, a_tiles, name):
            """a = silu(x Wg) for one column tile, all 8 f-blocks -> SBUF bf16."""
            for m in range(KB):
                pg = ppg.tile([P, NT], _F32, tag="pg")
                for k in range(KB):
                    nc.tensor.matmul(
                        pg[:, :cn], wslice(wg_t, k, m), x_row[k][ci],
                        start=(k == 0), stop=(k == KB - 1),
                    )
                sg = tmp.tile([P, NT], _F32, tag="sg")
                nc.scalar.activation(
                    out=sg[:, :cn], in_=pg[:, :cn],
                    func=mybir.ActivationFunctionType.Sigmoid,
                )
                a = res.tile([P, cn], _BF16, tag=f"a_{name}{m}_{ci % 2}")
                nc.vector.tensor_mul(a, pg[:, :cn], sg[:, :cn])
                a_tiles[m] = a

        def stage_a_pu(x_row, ci, cn, wu_t, a_tiles, s_tiles, name):
            """s = a * (x Wu) for one column tile -> SBUF bf16."""
            for m in range(KB):
                pu = ppu.tile([P, NT], _F32, tag="pu")
                for k in range(KB):
                    nc.tensor.matmul(
                        pu[:, :cn], wslice(wu_t, k, m), x_row[k][ci],
                        start=(k == 0), stop=(k == KB - 1),
                    )
                s = res.tile([P, cn], _BF16, tag=f"s_{name}{m}_{ci}")
                nc.vector.tensor_mul(s, a_tiles[m], pu[:, :cn])
                s_tiles[m][ci] = s

        def stage_b(ci, c0, cn, wd_t, s_tiles, y_dram, last=False):
            """y = s @ Wd (transposed) for one column tile, all 8 d-blocks."""
            yre = y_dram.rearrange("(m p) c -> p m c", p=P)
            for m in range(KB):
                po = ppo.tile([P, NT], _F32, tag="po")
                for k in range(KB):
                    nc.tensor.matmul(
                        po[:, :cn], wslice(wd_t, k, m), s_tiles[k][ci],
                        start=(k == 0), stop=(k == KB - 1),
                    )
                ot = outp.tile([P, NT], _BF16, tag="ot")
                nc.vector.tensor_copy(ot[:, :cn], po[:, :cn])
                if last and m == KB - 1:
                    # split the very last store 4 ways across both queue
                    # classes so its transfer doesn't serialize the tail
                    q = cn // 4
                    for j, eng in enumerate((nc.sync, nc.gpsimd, nc.sync, nc.gpsimd)):
                        a0 = j * q
                        a1 = (j + 1) * q if j < 3 else cn
                        eng.dma_start(
                            out=yre[:, m, c0 + a0 : c0 + a1], in_=ot[:, a0:a1]
                        )
                else:
                    nc.sync.dma_start(out=yre[:, m, c0 : c0 + cn], in_=ot[:, :cn])

        s_r = [[None] * len(cts_r) for _ in range(KB)]
        s_s = [[None] * len(cts_s) for _ in range(KB)]
        a_cur = [None] * KB

        # Order: all column tiles of routed+shared stage A except the last
        # routed tile, then their stage Bs, then the last routed tile's A+B —
        # so the only serial tail is one small tile's chain + a 4-way store.
        for i, (c0, cn) in enumerate(cts_r[:-1]):
            stage_a_pg(xt_sb, i, cn, wg_sb, a_cur, "r")
            stage_a_pu(xt_sb, i, cn, wu_sb, a_cur, s_r, "r")
        for i, (c0, cn) in enumerate(cts_s):
            stage_a_pg(xst_sb, i, cn, wgs_sb, a_cur, "s")
            stage_a_pu(xst_sb, i, cn, wus_sb, a_cur, s_s, "s")
        for i, (c0, cn) in enumerate(cts_r[:-1]):
            stage_b(i, c0, cn, wd_sb, s_r, yr)
        for i, (c0, cn) in enumerate(cts_s):
            stage_b(i, c0, cn, wds_sb, s_s, ys)
        i, (c0, cn) = len(cts_r) - 1, cts_r[-1]
        stage_a_pg(xt_sb, i, cn, wg_sb, a_cur, "r")
        stage_a_pu(xt_sb, i, cn, wu_sb, a_cur, s_r, "r")
        stage_b(i, c0, cn, wd_sb, s_r, yr, last=True)

    _split_multi_waits(nc)
    return nc


def _split_multi_waits(nc):
    """The bundled walrus lowers at most ONE sync wait per instruction (every
    instruction struct has a single EVENTS slot and codegen refuses to split).
    Tile emits multi-wait sync_infos, so hoist all but one wait onto
    InstEventSemaphore carriers inserted just before the instruction on the
    same engine queue — the sequencer blocks on the carriers first, which is
    strictly more conservative than the original multi-wait semantics."""
    f = nc.m.functions[0]
    for bb in f.blocks:
        insts = bb.instructions
        idx = 0
        while idx < len(insts):
            ins = insts[idx]
            si = ins.sync_info
            if si is not None and len(si.on_wait) > 1:
                waits = list(si.on_wait)
                keep = len(waits) - 1
                if isinstance(ins, mybir.InstDMACopy):
                    for j, w in enumerate(waits):
                        if w.ant_name and w.ant_name.startswith("DMA"):
                            keep = j
                            break
                carriers = []
                for j, w in enumerate(waits):
                    if j == keep:
                        continue
                    es = mybir.InstEventSemaphore(
                        name=nc.get_next_instruction_name(), ins=[], outs=[]
                    )
                    es.engine = ins.engine
                    es.sync_info = mybir.SyncInfo(on_wait=[w], on_update=[])
                    nc.register_instruction(es)
                    carriers.append(es)
                ins.sync_info = mybir.SyncInfo(
                    on_wait=[waits[keep]], on_update=list(si.on_update)
                )
                for c in reversed(carriers):
                    insts.insert(idx, c)
                idx += len(carriers)
            idx += 1


def route(xf, Wr, expert_bias, k):
    """Host router: replicates the reference routing math exactly (fp32)."""
    logits = xf @ Wr + expert_bias[None, :]
    gates = 1.0 / (1.0 + np.exp(-logits))
    # stable argsort matches jax.lax.top_k tie-breaking (lowest index first)
    order = np.argsort(-gates, axis=1, kind="stable")
    topk_idx = order[:, :k]
    topk_gates = np.take_along_axis(gates, topk_idx, axis=1)
    topk_gates = topk_gates / (topk_gates.sum(axis=1, keepdims=True) + 1e-9)
    return topk_idx, topk_gates


def prepare(x, Wg_s, Wu_s, Wd_s, Wg, Wu, Wd, Wr, expert_bias, top_k):
    """Host-side sharding: routing + per-expert gather + weight distribution."""
    bf16 = ml_dtypes.bfloat16
    x = np.asarray(x, np.float32)
    xf = x.reshape(-1, D)
    T = xf.shape[0]
    TS = T // NCORES
    k = int(top_k)

    topk_idx, topk_gates = route(
        xf, np.asarray(Wr, np.float32), np.asarray(expert_bias, np.float32), k
    )

    idx_e, w_e = [], []
    for e in range(E):
        mask = topk_idx == e
        rows = np.nonzero(mask.any(axis=1))[0]
        idx_e.append(rows)
        w_e.append((mask[rows] * topk_gates[rows]).sum(axis=1).astype(np.float32))
    n_e = [len(r) for r in idx_e]
    C, _, _ = _pad_c(max(P, max(n_e)))

    shared_w = {
        "wgs": np.asarray(Wg_s, np.float32).astype(bf16),
        "wus": np.asarray(Wu_s, np.float32).astype(bf16),
        "wds": np.asarray(Wd_s, np.float32).astype(bf16),
    }
    in_maps = []
    for e in range(E):
        xe = np.zeros((D, C), bf16)
        xe[:, : n_e[e]] = xf[idx_e[e]].T.astype(bf16)
        in_maps.append(
            {
                "xt": xe,
                "xst": np.ascontiguousarray(xf[e * TS : (e + 1) * TS].T).astype(bf16),
                "wg": np.asarray(Wg[e], np.float32).astype(bf16),
                "wu": np.asarray(Wu[e], np.float32).astype(bf16),
                "wd": np.asarray(Wd[e], np.float32).astype(bf16),
                **shared_w,
            }
        )
    return in_maps, idx_e, w_e, C, TS, x.shape


def combine(results, idx_e, w_e, out_shape):
    """Host-side unshard: weighted scatter-add of expert outputs + shared."""
    T = out_shape[0] * out_shape[1]
    out = np.zeros((T, D), np.float32)
    TS = T // NCORES
    for e in range(E):
        n = len(idx_e[e])
        out[idx_e[e]] += results[e]["yr"][:, :n].T.astype(np.float32) * w_e[e][:, None]
        out[e * TS : (e + 1) * TS] += results[e]["ys"].T.astype(np.float32)
    return out.reshape(out_shape)


def run_spmd(in_maps, C, TS, **kwargs):
    key = (C, TS)
    if key not in _nc_cache:
        _nc_cache[key] = build_bass(C, TS)
    return run_bass_kernel_spmd(
        _nc_cache[key], in_maps, core_ids=list(range(NCORES)), **kwargs
    )


def kernel(x, Wg_s, Wu_s, Wd_s, Wg, Wu, Wd, Wr, expert_bias, top_k):
    in_maps, idx_e, w_e, C, TS, out_shape = prepare(
        x, Wg_s, Wu_s, Wd_s, Wg, Wu, Wd, Wr, expert_bias, top_k
    )
    results = run_spmd(in_maps, C, TS).results
    out = combine(results, idx_e, w_e, out_shape)
    aux_loss = np.zeros((), dtype=np.float32)
    return out, aux_loss


# revision 35
# speedup vs baseline: 1.0385x; 1.0385x over previous
"""MoE layer (shared SwiGLU expert + 8 routed SwiGLU experts, sigmoid top-2
routing) on 8 Trainium2 NeuronCores.

Sharding strategy (expert-parallel, per the problem's sharding hint):
  - Router (x @ Wr, sigmoid, top-k, gate normalization) and the token->expert
    dispatch run on host as part of input sharding: core e receives exactly the
    tokens routed to expert e (capacity-padded so all 8 cores share one SPMD
    program), plus a 1/8 token shard for the replicated shared expert.
  - Each core holds only its own expert's weights (Wg[e], Wu[e], Wd[e]) plus
    the shared-expert weights, and computes two SwiGLU FFNs:
        routed: [C, D] tokens  -> silu(X Wg) * (X Wu) @ Wd
        shared: [T/8, D] shard -> same with shared weights
  - Host applies the top-k combine weights and scatter-adds expert outputs
    back to token positions (the "all-to-all return"), then adds the shared
    output.

Device kernel details:
  - Activations live transposed ([D, tokens]) so both FFN matmuls use the
    natural weight layout as the stationary operand; no on-device transposes.
  - bf16 weights/activations, fp32 PSUM accumulation, bf16 outputs.
  - Stage A is split into a Wg pass and a Wu pass per column tile with
    silu(hg) parked in SBUF between them, so the PE only needs Wg + the first
    x tile to start while Wu/Wd/shared weights stream in behind.
  - The shared FFN is split into two half tiles so its stage B overlaps
    stage A instead of serializing at the kernel tail.
  - The bundled walrus only lowers ONE sync wait per instruction; a post-pass
    hoists extra waits onto InstEventSemaphore carriers.
"""

import numpy as np
import ml_dtypes

import concourse.bass as bass
import concourse.mybir as mybir
import concourse.tile as tile
from concourse.bass_utils import run_bass_kernel_spmd

B, L, D, F, E = 2, 2048, 1024, 1024, 8
NCORES = 8
P = 128  # SBUF partitions
KB = D // P  # k-blocks per contraction (8)
NT = 512  # free-dim tile (one fp32 PSUM bank)

_BF16 = mybir.dt.bfloat16
_F32 = mybir.dt.float32

_nc_cache = {}


def _pad_c(c):
    # balanced split into minimum number of <=NT tiles, each a multiple of 8:
    # odd-width bf16 tiles stream through the PE at half rate (measured
    # 318ns vs 155ns per matmul), so pad the capacity up instead
    n = -(-c // NT)
    tile_w = -(-c // (n * 8)) * 8
    return tile_w * n, n, tile_w


def _ctiles(c, nt=NT):
    n = -(-c // nt)
    base, rem = divmod(c, n)
    out, c0 = [], 0
    for i in range(n):
        cn = base + (1 if i < rem else 0)
        out.append((c0, cn))
        c0 += cn
    return out


def build_bass(C, TS):
    """One SPMD program: two SwiGLU FFNs (routed capacity C, shared shard TS)."""
    from contextlib import ExitStack

    nc = bass.Bass()
    xt = nc.declare_dram_parameter("xt", [D, C], _BF16, isOutput=False)
    xst = nc.declare_dram_parameter("xst", [D, TS], _BF16, isOutput=False)
    wg = nc.declare_dram_parameter("wg", [D, F], _BF16, isOutput=False)
    wu = nc.declare_dram_parameter("wu", [D, F], _BF16, isOutput=False)
    wd = nc.declare_dram_parameter("wd", [F, D], _BF16, isOutput=False)
    wgs = nc.declare_dram_parameter("wgs", [D, F], _BF16, isOutput=False)
    wus = nc.declare_dram_parameter("wus", [D, F], _BF16, isOutput=False)
    wds = nc.declare_dram_parameter("wds", [F, D], _BF16, isOutput=False)
    yr = nc.declare_dram_parameter("yr", [D, C], _BF16, isOutput=True)
    ys = nc.declare_dram_parameter("ys", [D, TS], _BF16, isOutput=True)

    with tile.TileContext(nc) as tc, ExitStack() as ctx:
        res = ctx.enter_context(tc.tile_pool(name="resident", bufs=1))
        ppg = ctx.enter_context(tc.tile_pool(name="ppg", bufs=3, space="PSUM"))
        ppu = ctx.enter_context(tc.tile_pool(name="ppu", bufs=2, space="PSUM"))
        ppo = ctx.enter_context(tc.tile_pool(name="ppo", bufs=3, space="PSUM"))
        tmp = ctx.enter_context(tc.tile_pool(name="tmp", bufs=4))
        outp = ctx.enter_context(tc.tile_pool(name="outp", bufs=6))

        def load_w_half(dram_ap, name, half, engine):
            # one half-F tile per k-slice on the given engine's queue class
            n = dram_ap.shape[1]
            h = n // 2
            r = dram_ap.rearrange("(k p) n -> p k n", p=P)
            ts = []
            lo = half * h
            for k in range(KB):
                t = res.tile([P, h], _BF16, tag=f"{name}{k}{'ab'[half]}")
                engine.dma_start(out=t, in_=r[:, k, lo : lo + h])
                ts.append(t)
            return ts

        def load_w(dram_ap, name, split=False, split_engine=None):
            # [K*P, N] dram -> per-k tiles; split=True makes two half-F tiles
            # per k-slice (separate deps) so consumers of the first half can
            # start as soon as its 1MB lands; split_engine places the second
            # half on the other DMA queue class for extra early bandwidth.
            if split:
                a = load_w_half(dram_ap, name, 0, nc.sync)
                b = load_w_half(dram_ap, name, 1, split_engine or nc.sync)
                return list(zip(a, b))
            n = dram_ap.shape[1]
            r = dram_ap.rearrange("(k p) n -> p k n", p=P)
            ts = []
            for k in range(KB):
                t = res.tile([P, n], _BF16, tag=f"{name}{k}")
                nc.sync.dma_start(out=t, in_=r[:, k, :])
                ts.append((t,))
            return ts

        def wslice(ts, k, m):
            row = ts[k]
            if len(row) == 2:
                h = row[0].shape[-1]
                t = row[(m * P) // h]
                o = (m * P) % h
                return t[:, o : o + P]
            return row[0][:, m * P : (m + 1) * P]

        def load_x_ct(dram_ap, name, i, c0, cn, ts):
            # one column tile of x (all k slices) on the gpsimd queue class
            r = dram_ap.rearrange("(k p) n -> p k n", p=P)
            for k in range(KB):
                t = res.tile([P, cn], _BF16, tag=f"{name}{k}_{i}")
                nc.gpsimd.dma_start(out=t, in_=r[:, k, c0 : c0 + cn])
                ts[k][i] = t

        cts_r = _ctiles(C)
        cts_s = _ctiles(TS)
        xt_sb = [[None] * len(cts_r) for _ in range(KB)]
        xst_sb = [[None] * len(cts_s) for _ in range(KB)]

        # Emission order = per-queue-class consumption order. Each class
        # drains FIFO, so both classes deliver exactly what the PE needs
        # next: wg + first x tile first, wu split across BOTH classes, the
        # late-needed matrices (wd/shared) last.
        wg_sb = load_w(wg, "wg", split=True)
        load_x_ct(xt, "xt", 0, *cts_r[0], xt_sb)
        wu_a = load_w_half(wu, "wu", 0, nc.gpsimd)
        wu_b = load_w_half(wu, "wu", 1, nc.sync)
        wu_sb = list(zip(wu_a, wu_b))
        for i in range(1, min(2, len(cts_r))):
            load_x_ct(xt, "xt", i, *cts_r[i], xt_sb)
        for i in range(2, len(cts_r)):
            load_x_ct(xt, "xt", i, *cts_r[i], xt_sb)
        wgs_sb = load_w(wgs, "wgs")
        for i in range(len(cts_s)):
            load_x_ct(xst, "xst", i, *cts_s[i], xst_sb)
        wus_sb = load_w(wus, "wus")
        wd_sb = load_w(wd, "wd")
        wds_sb = load_w(wds, "wds")

        def stage_a_pg(x_row, ci, cn, wg_t, a_tiles, name):
            """a = silu(x Wg) for one column tile, all 8 f-blocks -> SBUF bf16."""
            for m in range(KB):
                pg = ppg.tile([P, NT], _F32, tag="pg")
                for k in range(KB):
                    nc.tensor.matmul(
                        pg[:, :cn], wslice(wg_t, k, m), x_row[k][ci],
                        start=(k == 0), stop=(k == KB - 1),
                    )
                sg = tmp.tile([P, NT], _F32, tag="sg")
                nc.scalar.activation(
                    out=sg[:, :cn], in_=pg[:, :cn],
                    func=mybir.ActivationFunctionType.Sigmoid,
                )
                a = res.tile([P, cn], _BF16, tag=f"a_{name}{m}_{ci % 2}")
                nc.vector.tensor_mul(a, pg[:, :cn], sg[:, :cn])
                a_tiles[m] = a

        def stage_a_pu(x_row, ci, cn, wu_t, a_tiles, s_tiles, name):
            """s = a * (x Wu) for one column tile -> SBUF bf16."""
            for m in range(KB):
                pu = ppu.tile([P, NT], _F32, tag="pu")
                for k in range(KB):
                    nc.tensor.matmul(
                        pu[:, :cn], wslice(wu_t, k, m), x_row[k][ci],
                        start=(k == 0), stop=(k == KB - 1),
                    )
                s = res.tile([P, cn], _BF16, tag=f"s_{name}{m}_{ci}")
                nc.vector.tensor_mul(s, a_tiles[m], pu[:, :cn])
                s_tiles[m][ci] = s

        def stage_b(ci, c0, cn, wd_t, s_tiles, y_dram, last=False):
            """y = s @ Wd (transposed) for one column tile, all 8 d-blocks."""
            yre = y_dram.rearrange("(m p) c -> p m c", p=P)
            for m in range(KB):
                po = ppo.tile([P, NT], _F32, tag="po")
                for k in range(KB):
                    nc.tensor.matmul(
                        po[:, :cn], wslice(wd_t, k, m), s_tiles[k][ci],
                        start=(k == 0), stop=(k == KB - 1),
                    )
                ot = outp.tile([P, NT], _BF16, tag="ot")
                nc.vector.tensor_copy(ot[:, :cn], po[:, :cn])
                if last and m == KB - 1:
                    # split the very last store 4 ways across both queue
                    # classes so its transfer doesn't serialize the tail
                    q = cn // 4
                    for j, eng in enumerate((nc.sync, nc.gpsimd, nc.sync, nc.gpsimd)):
                        a0 = j * q
                        a1 = (j + 1) * q if j < 3 else cn
                        eng.dma_start(
                            out=yre[:, m, c0 + a0 : c0 + a1], in_=ot[:, a0:a1]
                        )
                else:
                    nc.sync.dma_start(out=yre[:, m, c0 : c0 + cn], in_=ot[:, :cn])

        s_r = [[None] * len(cts_r) for _ in range(KB)]
        s_s = [[None] * len(cts_s) for _ in range(KB)]
        a_cur = [None] * KB

        # Order: all column tiles of routed+shared stage A except the last
        # routed tile, then their stage Bs, then the last routed tile's A+B —
        # so the only serial tail is one small tile's chain + a 4-way store.
        for i, (c0, cn) in enumerate(cts_r[:-1]):
            stage_a_pg(xt_sb, i, cn, wg_sb, a_cur, "r")
            stage_a_pu(xt_sb, i, cn, wu_sb, a_cur, s_r, "r")
        for i, (c0, cn) in enumerate(cts_s):
            stage_a_pg(xst_sb, i, cn, wgs_sb, a_cur, "s")
            stage_a_pu(xst_sb, i, cn, wus_sb, a_cur, s_s, "s")
        for i, (c0, cn) in enumerate(cts_r[:-1]):
            stage_b(i, c0, cn, wd_sb, s_r, yr)
        for i, (c0, cn) in enumerate(cts_s):
            stage_b(i, c0, cn, wds_sb, s_s, ys)
        i, (c0, cn) = len(cts_r) - 1, cts_r[-1]
        stage_a_pg(xt_sb, i, cn, wg_sb, a_cur, "r")
        stage_a_pu(xt_sb, i, cn, wu_sb, a_cur, s_r, "r")
        stage_b(i, c0, cn, wd_sb, s_r, yr, last=True)

    _split_multi_waits(nc)
    return nc


def _split_multi_waits(nc):
    """The bundled walrus lowers at most ONE sync wait per instruction (every
    instruction struct has a single EVENTS slot and codegen refuses to split).
    Tile emits multi-wait sync_infos, so hoist all but one wait onto
    InstEventSemaphore carriers inserted just before the instruction on the
    same engine queue — the sequencer blocks on the carriers first, which is
    strictly more conservative than the original multi-wait semantics."""
    f = nc.m.functions[0]
    for bb in f.blocks:
        insts = bb.instructions
        idx = 0
        while idx < len(insts):
            ins = insts[idx]
            si = ins.sync_info
            if si is not None and len(si.on_wait) > 1:
                waits = list(si.on_wait)
                keep = len(waits) - 1
                if isinstance(ins, mybir.InstDMACopy):
                    for j, w in enumerate(waits):
                        if w.ant_name and w.ant_name.startswith("DMA"):
                            keep = j
                            break
                carriers = []
                for j, w in enumerate(waits):
                    if j == keep:
                        continue
                    es = mybir.InstEventSemaphore(
                        name=nc.get_next_instruction_name(), ins=[], outs=[]
                    )
                    es.engine = ins.engine
                    es.sync_info = mybir.SyncInfo(on_wait=[w], on_update=[])
                    nc.register_instruction(es)
                    carriers.append(es)
                ins.sync_info = mybir.SyncInfo(
                    on_wait=[waits[keep]], on_update=list(si.on_update)
                )
                for c in reversed(carriers):
                    insts.insert(idx, c)
                idx += len(carriers)
            idx += 1


def route(xf, Wr, expert_bias, k):
    """Host router: replicates the reference routing math exactly (fp32)."""
    logits = xf @ Wr + expert_bias[None, :]
    gates = 1.0 / (1.0 + np.exp(-logits))
    # stable argsort matches jax.lax.top_k tie-breaking (lowest index first)
    order = np.argsort(-gates, axis=1, kind="stable")
    topk_idx = order[:, :k]
    topk_gates = np.take_along_axis(gates, topk_idx, axis=1)
    topk_gates = topk_gates / (topk_gates.sum(axis=1, keepdims=True) + 1e-9)
    return topk_idx, topk_gates


def prepare(x, Wg_s, Wu_s, Wd_s, Wg, Wu, Wd, Wr, expert_bias, top_k):
    """Host-side sharding: routing + per-expert gather + weight distribution."""
    bf16 = ml_dtypes.bfloat16
    x = np.asarray(x, np.float32)
    xf = x.reshape(-1, D)
    T = xf.shape[0]
    TS = T // NCORES
    k = int(top_k)

    topk_idx, topk_gates = route(
        xf, np.asarray(Wr, np.float32), np.asarray(expert_bias, np.float32), k
    )

    idx_e, w_e = [], []
    for e in range(E):
        mask = topk_idx == e
        rows = np.nonzero(mask.any(axis=1))[0]
        idx_e.append(rows)
        w_e.append((mask[rows] * topk_gates[rows]).sum(axis=1).astype(np.float32))
    n_e = [len(r) for r in idx_e]
    C, _, _ = _pad_c(max(P, max(n_e)))

    shared_w = {
        "wgs": np.asarray(Wg_s, np.float32).astype(bf16),
        "wus": np.asarray(Wu_s, np.float32).astype(bf16),
        "wds": np.asarray(Wd_s, np.float32).astype(bf16),
    }
    in_maps = []
    for e in range(E):
        xe = np.zeros((D, C), bf16)
        xe[:, : n_e[e]] = xf[idx_e[e]].T.astype(bf16)
        in_maps.append(
            {
                "xt": xe,
                "xst": np.ascontiguousarray(xf[e * TS : (e + 1) * TS].T).astype(bf16),
                "wg": np.asarray(Wg[e], np.float32).astype(bf16),
                "wu": np.asarray(Wu[e], np.float32).astype(bf16),
                "wd": np.asarray(Wd[e], np.float32).astype(bf16),
                **shared_w,
            }
        )
    return in_maps, idx_e, w_e, C, TS, x.shape


def combine(results, idx_e, w_e, out_shape):
    """Host-side unshard: weighted scatter-add of expert outputs + shared."""
    T = out_shape[0] * out_shape[1]
    out = np.zeros((T, D), np.float32)
    TS = T // NCORES
    for e in range(E):
        n = len(idx_e[e])
        out[idx_e[e]] += results[e]["yr"][:, :n].T.astype(np.float32) * w_e[e][:, None]
        out[e * TS : (e + 1) * TS] += results[e]["ys"].T.astype(np.float32)
    return out.reshape(out_shape)


def run_spmd(in_maps, C, TS, **kwargs):
    key = (C, TS)
    if key not in _nc_cache:
        _nc_cache[key] = build_bass(C, TS)
    return run_bass_kernel_spmd(
        _nc_cache[key], in_maps, core_ids=list(range(NCORES)), **kwargs
    )


def kernel(x, Wg_s, Wu_s, Wd_s, Wg, Wu, Wd, Wr, expert_bias, top_k):
    in_maps, idx_e, w_e, C, TS, out_shape = prepare(
        x, Wg_s, Wu_s, Wd_s, Wg, Wu, Wd, Wr, expert_bias, top_k
    )
    results = run_spmd(in_maps, C, TS).results
    out = combine(results, idx_e, w_e, out_shape)
    aux_loss = np.zeros((), dtype=np.float32)
    return out, aux_loss
